# revision 1
# baseline (speedup 1.0000x reference)
"""Conv7x7(SAME) + LIF scan kernel for Trainium2, 8 NeuronCores.

Strategy:
- Shard H=512 spatially: core c owns output rows [64c, 64c+64). Host passes
  each core its 70-row input slab (64 + 3-row halo each side, zero padded),
  so no device-to-device communication is needed.
- Conv: 7x7 fp32 conv as 7 banded matmuls on the TensorEngine (band = the 7
  row-taps for one column-shift dx; column shifts realized as free-dim offsets
  into a width-padded SBUF tile). PSUM accumulates over dx. The two width
  halves run as col-tiled matmul pairs (tile_position) so M=64 doesn't waste
  the 128-wide PE array; output lands natively as [128, 256] = (half, row) x
  colchunk.
- LIF: bit-exact replication of the reference's per-op fp32 arithmetic on the
  VectorEngine: 6 ops per timestep on [128, 256] tiles.
    s = (i * 0.1) - i          (= -i_dec, exact negation)
    d = i - v
    v = (d * 0.1) + v          (= v_dec)
    z = (v - 1.0) > 0
    v = 0 where z              (copy_predicated reset)
    i = x_t - s                (= i_dec + x_t bitwise)
"""
import numpy as np
import concourse.bacc as bacc
import concourse.mybir as mybir
import concourse.tile as tile
from concourse.bass_utils import run_bass_kernel_spmd

T, H, WD, KK, PAD = 128, 512, 512, 7, 3
NCORES = 8
ROWS = H // NCORES            # 64 output rows per core
KP = ROWS + 2 * PAD           # 70 input rows per core
XB = 8                        # x tile buffers
ZB = 8                        # z staging buffers
NPS = 8                       # psum tiles in flight (PSUM = 8 banks)

_cached = None


def _build():
    global _cached
    if _cached is not None:
        return _cached

    f32 = mybir.dt.float32
    u32 = mybir.dt.uint32
    Alu = mybir.AluOpType

    nc = bacc.Bacc("TRN2", debug=False, num_devices=NCORES)
    xs_d = nc.dram_tensor("xs", (T, KP, WD), f32, kind="ExternalInput")
    bm_d = nc.dram_tensor("bm", (KP, KK * ROWS), f32, kind="ExternalInput")
    zs_d = nc.dram_tensor("zs", (T, ROWS, WD), f32, kind="ExternalOutput")

    with tile.TileContext(nc) as tc:
        with (
            tc.tile_pool(name="pool", bufs=1) as pool,
            tc.tile_pool(name="psum", bufs=1, space="PSUM") as psum,
        ):
            bm_t = pool.tile([KP, KK * ROWS], f32)
            nc.gpsimd.dma_start(bm_t[:], bm_d.ap())

            xts = [pool.tile([KP, WD + 2 * PAD], f32, name=f"xt{i}")
                   for i in range(XB)]
            for xt in xts:
                nc.gpsimd.memset(xt[:], 0.0)

            zts = [pool.tile([128, 256], f32, name=f"zt{i}") for i in range(ZB)]
            pss = [psum.tile([128, 256], f32, name=f"ps{i}") for i in range(NPS)]

            v_t = pool.tile([128, 256], f32)
            i_t = pool.tile([128, 256], f32)
            d_t = pool.tile([128, 256], f32)
            s_t = pool.tile([128, 256], f32)
            zero_t = pool.tile([128, 256], f32)
            nc.gpsimd.memset(v_t[:], 0.0)
            nc.gpsimd.memset(i_t[:], 0.0)
            nc.gpsimd.memset(zero_t[:], 0.0)

            for t in range(T):
                xt = xts[t % XB]
                nc.sync.dma_start(xt[:, PAD:PAD + WD], xs_d.ap()[t])
                ps = pss[t % NPS]
                for dx in range(KK):
                    for h in range(2):
                        nc.tensor.matmul(
                            ps[h * 64:(h + 1) * 64, :],
                            bm_t[:, dx * ROWS:(dx + 1) * ROWS],
                            xt[:, h * 256 + dx: h * 256 + dx + 256],
                            start=(dx == 0), stop=(dx == KK - 1),
                            tile_position=(0, h * 64),
                        )
                z_t = zts[t % ZB]
                # LIF step (all DVE, bit-exact vs reference order)
                nc.vector.scalar_tensor_tensor(
                    s_t[:], i_t[:], 0.1, i_t[:], Alu.mult, Alu.subtract)
                nc.vector.tensor_tensor(d_t[:], i_t[:], v_t[:], Alu.subtract)
                # psum read happens early so the bank frees for t+NPS
                nc.vector.tensor_tensor(i_t[:], ps[:], s_t[:], Alu.subtract)
                nc.vector.scalar_tensor_tensor(
                    v_t[:], d_t[:], 0.1, v_t[:], Alu.mult, Alu.add)
                nc.vector.tensor_scalar(
                    z_t[:], v_t[:], 1.0, 0.0, Alu.subtract, Alu.is_gt)
                nc.vector.copy_predicated(v_t[:], z_t[:].bitcast(u32), zero_t[:])

                nc.sync.dma_start(
                    zs_d.ap()[t].rearrange("r (h n) -> h r n", h=2), z_t[:])

    nc.compile()
    _cached = nc
    return nc


def _build_bmats(W):
    """bm[k, dx*64 + m] = W[dy=k-m, dx] for 0 <= k-m <= 6."""
    W = np.asarray(W, np.float32).reshape(KK, KK)
    bm = np.zeros((KP, KK * ROWS), np.float32)
    for dx in range(KK):
        for m in range(ROWS):
            for dy in range(KK):
                bm[m + dy, dx * ROWS + m] = W[dy, dx]
    return bm


def kernel(x, W):
    x = np.asarray(x, np.float32)
    nc = _build()
    bm = _build_bmats(W)
    xp = np.pad(x[:, 0], ((0, 0), (PAD, PAD), (0, 0)))  # [T, H+6, W]
    in_maps = []
    for c in range(NCORES):
        shard = np.ascontiguousarray(xp[:, c * ROWS: c * ROWS + KP, :])
        in_maps.append({"xs": shard, "bm": bm})
    res = run_bass_kernel_spmd(nc, in_maps, core_ids=list(range(NCORES)))
    z = np.concatenate([r["zs"] for r in res.results], axis=1)  # [T, H, W]
    return z.reshape(T, 1, H, WD).astype(np.float32)

